# revision 32
# baseline (speedup 1.0000x reference)
"""Contrastive loss (NT-Xent) on 8 Trainium2 NeuronCores.

Row-parallel over the [2B, 2B] similarity matrix: core c computes rows
[c*1024, (c+1)*1024). Inputs are passed host-transposed ([D, 2B]) in bf16 with
the column blocks rotated per core so diagonal / positive blocks land at fixed
tile indices on every core (uniform SPMD program).

Matmuls run in fp8e4 DoubleRow mode (2 MACs/cell/cycle) on RAW features:
contraction pairs are packed as [128, 2, X] tiles. Neither normalization gates
the PE — the row-side 1/norm folds into the ACT exp's per-partition scale
vector, and the column-side 1/norm either pre-scales the rhs (late tiles) or
is applied after the matmul to SBUF-stashed raw sims ("bridge" tiles that run
while the inverse-norm AllGather and its one-time CC barrier are in flight;
the column scale commutes out of the matmul). Row-wise logsumexp uses the ACT
engine's fused accumulate over 2-PSUM-bank [128,1024] tiles. Per-core partial
sums are combined with a scalar AllGather.
"""

import os
import sys

for _p in ("/opt/trn_rl_repo", "/root/.axon_site/_ro/trn_rl_repo"):
    if os.path.isdir(_p) and _p not in sys.path:
        sys.path.append(_p)

import numpy as np

B = 4096
D = 1024
TWO_B = 2 * B
TEMP = 0.07
N_CORES = 8
BLK = TWO_B // N_CORES  # 1024 columns per block
KP = D // 256  # 4 contraction pair-chunks of 2x128
NT2 = N_CORES  # 8 double-column tiles of 1024 == rotated block index
MT = BLK // 128  # 8 row slabs of 128
FSC = 8.0  # column-side pre-scale folded into inv norms (fp8 range use)
RSCALE = 1.0 / (FSC * TEMP)  # row-side inv-norm * this = exp scale

# Rotated blocks 1..N_BRIDGE run as raw-matmul "bridge" tiles while the
# inv-norm AllGather is in flight (block 0 always does). Block 4 holds the
# positives; when bridged they are extracted from the stashed raw sims.
N_BRIDGE = int(os.environ.get("CL_BRIDGE", 4))
BRIDGE = set(range(1, 1 + N_BRIDGE))

_cache = {}


def _build():
    import concourse.bass as bass
    import concourse.bacc as bacc
    import concourse.mybir as mybir
    from concourse.tile import TileContext

    f32 = mybir.dt.float32
    bf16 = mybir.dt.bfloat16
    f8 = mybir.dt.float8e4
    AF = mybir.ActivationFunctionType
    ALU = mybir.AluOpType
    AX = mybir.AxisListType
    DR = mybir.MatmulPerfMode.DoubleRow

    nc = bacc.Bacc(None, target_bir_lowering=False, debug=False)
    ft = nc.dram_tensor("ft", [D, TWO_B], bf16, kind="ExternalInput")
    bsel = nc.dram_tensor("bsel", [8, 8 * 128], bf16, kind="ExternalInput")
    ident = nc.dram_tensor("ident", [128, 128], f32, kind="ExternalInput")
    esel = nc.dram_tensor("esel", [8, 8 * 128], f32, kind="ExternalInput")
    maskinv = nc.dram_tensor("maskinv", [128, 128], bf16, kind="ExternalInput")
    loss = nc.dram_tensor("loss", [1, 1], f32, kind="ExternalOutput")

    def pair_src(kk, c0, w):
        # DRAM view [128, 2, w]: [p, j, c] = ft[256*kk + 128*j + p, c0 + c]
        return ft[256 * kk : 256 * (kk + 1), c0 : c0 + w].rearrange(
            "(j p) c -> p j c", j=2
        )

    with TileContext(nc) as tc:
        with (
            tc.tile_pool(name="own", bufs=KP) as pool_own,
            tc.tile_pool(name="own8", bufs=KP) as pool_own8,
            tc.tile_pool(name="sq", bufs=KP) as pool_sq,
            tc.tile_pool(name="binv", bufs=8) as pool_binv,
            tc.tile_pool(name="raw", bufs=12) as pool_raw,
            tc.tile_pool(name="rhs8", bufs=14) as pool_rhs8,
            tc.tile_pool(name="sim", bufs=max(8 * len(BRIDGE), 1)) as pool_sim,
            tc.tile_pool(name="exp", bufs=5) as pool_e,
            tc.tile_pool(name="small", bufs=1) as pool_small,
            tc.tile_pool(name="junk", bufs=2) as pool_junk,
            tc.tile_pool(name="ps2", bufs=3, space="PSUM") as psum2,
            tc.tile_pool(name="ps1", bufs=2, space="PSUM") as psum1,
            tc.tile_pool(name="dram", bufs=4, space="DRAM") as dram,
        ):
            inv_in = dram.tile([1, BLK], f32, name="inv_in")
            inv_out = dram.tile([8, BLK], f32, name="inv_out")
            part_in = dram.tile([1, 1], f32, name="part_in")
            part_red = dram.tile([1, 1], f32, name="part_red")

            # --- constants ---
            ones_f = pool_small.tile([128, 1], f32, name="ones_f", tag="ones_f")
            nc.vector.memset(ones_f[:], 1.0)
            ones_r = pool_small.tile([128, 1], bf16, name="ones_r", tag="ones_r")
            nc.vector.tensor_copy(ones_r[:], ones_f[:])
            ident_sb = pool_small.tile([128, 128], f32, name="ident", tag="ident")
            nc.sync.dma_start(out=ident_sb[:], in_=ident[:])
            esel_sb = pool_small.tile([8, 8 * 128], f32, name="esel", tag="esel")
            nc.sync.dma_start(out=esel_sb[:], in_=esel[:])
            maskinv_sb = pool_small.tile([128, 128], bf16, name="maskinv", tag="maskinv")
            nc.sync.dma_start(out=maskinv_sb[:], in_=maskinv[:])
            bsel_sb = pool_small.tile([8, 8 * 128], bf16, name="bsel", tag="bsel")
            nc.sync.dma_start(out=bsel_sb[:], in_=bsel[:])

            # --- own block: load pair tiles, square (for norms), cast to raw
            # fp8 (the stationary operand for every matmul; also the rhs for
            # tile 0) ---
            own_raw = []
            own8 = []
            sq = []
            for kk in range(KP):
                t = pool_own.tile([128, 2, BLK], bf16, name="own_raw", tag="own_raw")
                nc.sync.dma_start(out=t[:], in_=pair_src(kk, 0, BLK))
                own_raw.append(t)
                s = pool_sq.tile([128, 2, BLK], bf16, name="sq", tag="sq")
                nc.vector.tensor_mul(s[:], t[:], t[:])
                sq.append(s)
            for kk in range(KP):
                t8 = pool_own8.tile([128, 2, BLK], f8, name="own8", tag="own8")
                nc.vector.tensor_copy(t8[:], own_raw[kk][:])
                own8.append(t8)

            # fold the squared pair tiles (DVE) so only 8 column-sum
            # matmuls are needed; column sums land directly in [128, MT]
            # partition layout (sqv stationary), so no single-lane
            # reciprocal or DMA transpose
            nc.vector.tensor_add(sq[0][:], sq[0][:], sq[1][:])
            nc.vector.tensor_add(sq[2][:], sq[2][:], sq[3][:])
            nc.vector.tensor_add(sq[0][:], sq[0][:], sq[2][:])
            sqv = pool_small.tile([128, BLK], bf16, name="sqv", tag="sqv")
            nc.vector.tensor_add(sqv[:], sq[0][:, 0, :], sq[0][:, 1, :])
            pN = psum1.tile([128, 512], f32, name="ps1", tag="ps1")
            for c in range(MT):
                nc.tensor.matmul(
                    pN[:, c : c + 1],
                    sqv[:, c * 128 : (c + 1) * 128],
                    ones_r[:],
                    start=True,
                    stop=True,
                )
            nrmT = pool_small.tile([128, MT], f32, name="nrmT", tag="nrmT")
            # sqrt(ss/64) = norm/8  ->  reciprocal = 8/norm
            nc.scalar.activation(
                nrmT[:], pN[:, 0:MT], AF.Sqrt, scale=1.0 / (FSC * FSC)
            )
            invT = pool_small.tile([128, MT], f32, name="invT", tag="invT")
            nc.vector.reciprocal(invT[:], nrmT[:])  # = 8/norm, [p, m] layout
            rower = pool_small.tile([128, MT], f32, name="rowexp", tag="rowexp")
            # exp scale = inv_row/(8*T) = invT/(64*T)
            nc.vector.tensor_scalar_mul(rower[:], invT[:], RSCALE / FSC)

            # free-layout inv via one PE transpose: T8f[m, p] = inv[m*128+p]
            pT = psum1.tile([128, 512], f32, name="ps1", tag="ps1")
            nc.tensor.transpose(pT[0:MT, 0:128], invT[:], ident_sb[:])
            t8f = pool_small.tile([8, 128], f32, name="t8f", tag="t8f")
            nc.vector.tensor_copy(t8f[:], pT[0:MT, 0:128])
            nc.scalar.dma_start(
                out=inv_in[:].rearrange("a (m c) -> m (a c)", m=8), in_=t8f[:]
            )
            nc.gpsimd.collective_compute(
                "AllGather",
                mybir.AluOpType.bypass,
                ins=[inv_in.opt()],
                outs=[inv_out.opt()],
                replica_groups=[list(range(N_CORES))],
            )

            # own-block column inverse norms (pre-AllGather): broadcast each
            # 128-col chunk of T8f across partitions via selector matmuls
            binv_own = pool_binv.tile([128, BLK], bf16, name="binv_own", tag="binv_own")
            for h in range(2):
                pb = psum1.tile([128, 512], f32, name="ps1", tag="ps1")
                for q in range(4):
                    m = h * 4 + q
                    nc.tensor.matmul(
                        pb[:, q * 128 : (q + 1) * 128],
                        esel_sb[:, m * 128 : (m + 1) * 128],
                        t8f[:],
                        start=True,
                        stop=True,
                    )
                nc.vector.tensor_copy(binv_own[:, h * 512 : (h + 1) * 512], pb[:])

            # --- accumulators ---
            rs_buf = pool_small.tile([128, MT * NT2], f32, name="rs_buf", tag="rs_buf")
            nc.vector.memset(rs_buf[:], 1.0)
            pos_all = pool_small.tile([128, MT], f32, name="pos_all", tag="pos_all")
            nc.vector.memset(pos_all[:], 0.0)

            sim_tiles = {}
            binv_rem = {}

            def mm_group(ps, rhs8, m):
                for h in range(2):
                    for kk in range(KP):
                        nc.tensor.matmul(
                            ps[:, h * 512 : (h + 1) * 512],
                            own8[kk][:, :, m * 128 : (m + 1) * 128],
                            rhs8[kk][h],
                            start=(kk == 0),
                            stop=(kk == KP - 1),
                            perf_mode=DR,
                        )

            max_np2 = int(os.environ.get("CL_MAXNP2", NT2))

            def do_tile(np2, ms=None):
                bridged = np2 in BRIDGE
                if np2 == 0:
                    rhs8 = [
                        [own8[kk][:, :, h * 512 : (h + 1) * 512] for h in range(2)]
                        for kk in range(KP)
                    ]
                else:
                    rhs8 = []
                    for kk in range(KP):
                        row = []
                        for h in range(2):
                            raw = pool_raw.tile(
                                [128, 2, 512], bf16, name="raw", tag="raw"
                            )
                            nc.sync.dma_start(
                                out=raw[:],
                                in_=pair_src(kk, np2 * BLK + h * 512, 512),
                            )
                            r8 = pool_rhs8.tile(
                                [128, 2, 512], f8, name="r8", tag="r8"
                            )
                            if bridged:
                                nc.vector.tensor_copy(r8[:], raw[:])
                            else:
                                for j in range(2):
                                    nc.vector.tensor_mul(
                                        r8[:, j, :],
                                        raw[:, j, :],
                                        binv_rem[np2][:, h * 512 : (h + 1) * 512],
                                    )
                            row.append(r8[:])
                        rhs8.append(row)
                for m in ms if ms is not None else range(MT):
                    ps = psum2.tile([128, 2 * 512], f32, name="ps2", tag="ps2")
                    mm_group(ps, rhs8, m)
                    if bridged:
                        sb = pool_sim.tile([128, 1024], bf16, name="sim", tag="sim")
                        nc.vector.tensor_copy(sb[:], ps[:])
                        sim_tiles[(np2, m)] = sb
                    elif np2 == 0:
                        # diagonal tile: inline column scale (binv_own is ready
                        # pre-AllGather), exp, zero self-sim, reduce on DVE
                        sm = pool_e.tile([128, 1024], bf16, name="e", tag="e")
                        nc.vector.tensor_mul(sm[:], ps[:], binv_own[:])
                        e = pool_e.tile([128, 1024], bf16, name="e", tag="e")
                        nc.scalar.activation(
                            e[:], sm[:], AF.Exp, scale=rower[:, m : m + 1]
                        )
                        nc.vector.tensor_mul(
                            e[:, m * 128 : (m + 1) * 128],
                            e[:, m * 128 : (m + 1) * 128],
                            maskinv_sb[:],
                        )
                        nc.vector.reduce_sum(
                            out=rs_buf[:, m * NT2 : m * NT2 + 1], in_=e[:], axis=AX.X
                        )
                    else:
                        e = pool_e.tile([128, 1024], bf16, name="e", tag="e")
                        nc.scalar.activation(
                            e[:],
                            ps[:],
                            AF.Exp,
                            scale=rower[:, m : m + 1],
                            accum_out=rs_buf[:, m * NT2 + np2 : m * NT2 + np2 + 1],
                        )

            def defer_one(np2, m, eng):
                # one stashed slab: column scale (DVE or idle GpSimd) + exp
                sb = sim_tiles[(np2, m)]
                sm = pool_e.tile([128, 1024], bf16, name="e", tag="e")
                eng.tensor_mul(sm[:], sb[:], binv_rem[np2][:])
                e = pool_e.tile([128, 1024], bf16, name="e", tag="e")
                nc.scalar.activation(
                    e[:],
                    sm[:],
                    AF.Exp,
                    scale=rower[:, m : m + 1],
                    accum_out=rs_buf[:, m * NT2 + np2 : m * NT2 + np2 + 1],
                )
                if np2 == 4:
                    junk = pool_junk.tile(
                        [128, 128], bf16, name="junkb", tag="junkb"
                    )
                    nc.vector.tensor_mul(
                        junk[:], sm[:, m * 128 : (m + 1) * 128], ident_sb[:]
                    )
                    nc.vector.reduce_sum(
                        out=pos_all[:, m : m + 1], in_=junk[:], axis=AX.X
                    )

            # tile 0: PE continues straight from the norm matmuls
            do_tile(0)

            def make_binv(np2):
                t = pool_binv.tile([128, BLK], bf16, name="binv_rem", tag="binv_rem")
                for h in range(2):
                    pr = psum1.tile([128, 512], f32, name="ps1", tag="ps1")
                    nc.tensor.matmul(
                        pr[:],
                        bsel_sb[:, np2 * 128 : (np2 + 1) * 128],
                        g_inv_r[:, h * 512 : (h + 1) * 512],
                        start=True,
                        stop=True,
                    )
                    nc.vector.tensor_copy(t[:, h * 512 : (h + 1) * 512], pr[:])
                binv_rem[np2] = t

            # --- bridge tiles: matmul raw columns while the AllGather runs ---
            stash = [np2 for np2 in sorted(BRIDGE) if np2 < max_np2]
            apath = [j for j in range(1, NT2) if j not in BRIDGE]
            for np2 in stash:
                do_tile(np2)

            # --- post-AllGather: per-block column inverse-norm broadcasts,
            # A-path blocks first ---
            g_inv = pool_small.tile([8, BLK], f32, name="g_inv", tag="g_inv")
            nc.sync.dma_start(out=g_inv[:], in_=inv_out[:])
            g_inv_r = pool_small.tile([8, BLK], bf16, name="g_inv_r", tag="g_inv_r")
            nc.vector.tensor_copy(g_inv_r[:], g_inv[:])
            for np2 in apath + stash:
                make_binv(np2)

            # interleave the deferred bridge backlog with the remaining
            # A-path tiles, a chunk of slabs per tile, so it drains while the
            # PE is still streaming. Early chunks lean on the idle GpSimd so
            # the DVE keeps feeding the A-path rhs normalize; late chunks go
            # DVE (faster) once the rhs work is done.
            defq = [(np2, m) for np2 in stash for m in range(MT)]
            n_ap = max(len(apath), 1)
            chunk = (len(defq) + n_ap - 1) // n_ap
            for i, np2 in enumerate(apath):
                if np2 < max_np2:
                    do_tile(np2)
                for k in range(chunk if i < n_ap - 1 else len(defq)):
                    if not defq:
                        break
                    dnp2, dm = defq.pop(0)
                    eng = nc.gpsimd if (i == 0 and k % 3 != 2) or (
                        i > 0 and k % 2 == 0
                    ) else nc.vector
                    defer_one(dnp2, dm, eng)

            # --- logsumexp + loss ---
            rs_all = pool_small.tile([128, MT], f32, name="rs_all", tag="rs_all")
            for m in range(MT):
                nc.vector.reduce_sum(
                    out=rs_all[:, m : m + 1],
                    in_=rs_buf[:, m * NT2 : (m + 1) * NT2],
                    axis=AX.X,
                )
            lse = pool_small.tile([128, MT], f32, name="lse", tag="lse")
            nc.scalar.activation(lse[:], rs_all[:], AF.Ln)
            poss = pool_small.tile([128, MT], f32, name="poss", tag="poss")
            nc.vector.tensor_mul(poss[:], pos_all[:], rower[:])
            diff = pool_small.tile([128, MT], f32, name="diff", tag="diff")
            nc.vector.tensor_sub(diff[:], lse[:], poss[:])
            dsum = pool_small.tile([128, 1], f32, name="dsum", tag="dsum")
            nc.vector.reduce_sum(out=dsum[:], in_=diff[:], axis=AX.X)
            pf = psum1.tile([128, 512], f32, name="ps1", tag="ps1")
            nc.tensor.matmul(pf[0:1, 0:1], dsum[:], ones_f[:], start=True, stop=True)
            part_sb = pool_small.tile([1, 1], f32, name="part_sb", tag="part_sb")
            nc.vector.tensor_copy(part_sb[:], pf[0:1, 0:1])
            nc.sync.dma_start(out=part_in[:], in_=part_sb[:])
            nc.gpsimd.collective_compute(
                "AllReduce",
                mybir.AluOpType.add,
                ins=[part_in.opt()],
                outs=[part_red.opt()],
                replica_groups=[list(range(N_CORES))],
            )
            back = pool_small.tile([1, 1], f32, name="back", tag="back")
            nc.sync.dma_start(out=back[:], in_=part_red[:])
            lout = pool_small.tile([1, 1], f32, name="lout", tag="lout")
            nc.scalar.mul(lout[:], back[:], 1.0 / TWO_B)
            nc.sync.dma_start(out=loss[:], in_=lout[:])

    nc.compile()
    return nc


def make_in_maps(features_1: np.ndarray, features_2: np.ndarray):
    import ml_dtypes

    f1 = np.asarray(features_1, dtype=np.float32)
    f2 = np.asarray(features_2, dtype=np.float32)
    f = np.concatenate([f1, f2], axis=0)  # [2B, D]
    ftb = (
        np.ascontiguousarray(f.T).astype(ml_dtypes.bfloat16).reshape(D, N_CORES, BLK)
    )

    ident = np.eye(128, dtype=np.float32)
    maskinv = (1.0 - np.eye(128, dtype=np.float32)).astype(ml_dtypes.bfloat16)
    esel = np.repeat(np.eye(8, dtype=np.float32), 128, axis=1)

    in_maps = []
    for c in range(N_CORES):
        order = [(c + j) % N_CORES for j in range(N_CORES)]
        ft_c = np.ascontiguousarray(ftb[:, order, :]).reshape(D, TWO_B)
        perm_c = np.zeros((8, 8), dtype=np.float32)
        for j in range(N_CORES):
            perm_c[(c + j) % N_CORES, j] = 1.0
        bsel_c = np.repeat(perm_c, 128, axis=1).astype(ml_dtypes.bfloat16)
        in_maps.append(
            {
                "ft": ft_c,
                "bsel": bsel_c,
                "ident": ident,
                "esel": esel,
                "maskinv": maskinv,
            }
        )
    return in_maps


def kernel(features_1: np.ndarray, features_2: np.ndarray) -> np.ndarray:
    from concourse.bass_utils import run_bass_kernel_spmd

    if "nc" not in _cache:
        _cache["nc"] = _build()
    nc = _cache["nc"]

    in_maps = make_in_maps(features_1, features_2)
    res = run_bass_kernel_spmd(nc, in_maps, list(range(N_CORES)))
    out = res.results[0]["loss"]
    return np.float32(out.reshape(()))
